# revision 1
# baseline (speedup 1.0000x reference)
# Trainium2 Bass kernel for nn_Democracy_loss (supervised-contrastive loss).
#
# Strategy: the dominant cost is the first embed GEMM
#   h_pre = X @ W1,  X: [320, 120000] f32, W1: [120000, 128] f32
# (215 MB of input read; everything downstream is tiny). We shard the
# CONTRACTION dim K=120000 across the 8 cores (15000 rows each) so W1 is
# *not* replicated: every input byte is read exactly once (~27 MB/core).
# Each core computes a partial h_pre^T = W1_c^T @ X_c^T into PSUM
# ([128, 320] f32, fits one bank) and returns it. The host sums the 8
# partials, applies b1/relu, the tiny 320x128x128 second GEMM, and the
# data-dependent ragged pos/neg loss grouping (integer metadata, host-side).
#
# Device layout: per core one packed DRAM input [128, 118, 448] where
# packed[p, t, 0:320]  = X^T[k0 + t*128 + p, :]   (moving operand tile)
# packed[p, t, 320:448] = W1[k0 + t*128 + p, :]   (stationary operand tile)
# so each chunk of k-tiles is ONE contiguous-per-partition dma_start
# (~2.3 MB), and each k-tile is one matmul:
#   psum[n, m] += lhsT(W1-tile [128k,128n]).T @ rhs(XT-tile [128k,320m])
# K per core = 15000, zero-padded to 118*128 = 15104.

import sys

import numpy as np

for _p in ("/opt/trn_rl_repo",):
    if _p not in sys.path:
        sys.path.append(_p)

NF, NC_SAMPLES, B_TOTAL = 256, 64, 320
IN_DIM = 120000
HID = 128
N_CORES = 8
K_PER_CORE = IN_DIM // N_CORES          # 15000
KTILES = (K_PER_CORE + 127) // 128      # 118 (padded to 15104)
K_PAD = KTILES * 128
NK_CHUNK = 10
# Small first chunk -> PE starts ~1.5 us earlier; small last chunk -> the
# final matmuls decouple from a full-size DMA. Measured min 48.9 / median
# 49.5 us vs 50.0/50.5 for uniform [10]*11+[8] (5 interleaved-phase samples).
_CHUNKS = [4] + [NK_CHUNK] * 11 + [4]   # 118 k-tiles
assert sum(_CHUNKS) == KTILES
PACK_W = B_TOTAL + HID                  # 448
# float32 (2-pass exact matmul): measured best. float32r (1-pass, ~1e-4 h_pre
# error) made DMA demand continuous and hit the per-core HBM fair-share cap
# with no end-to-end gain.
MM_F32R = False
# Interleaved A/B (6 samples each, drift-cancelled): bufs=4 and bufs=6 tie on
# min (~49.7 vs 50.0 us) but bufs=6 is much more consistent (median 50.5 vs
# 52.9) — better expected value for a single run.
IO_BUFS = 6
# PACK_DTYPE: "float32" (exact, ~27 MB/core, measured 86.5 us) or "float16"
# (halves DMA bytes, measured 49.3 us at 1.5e-6 final rel err; W1 is
# pre-scaled by 2^8 so its sigma=0.003 values stay in fp16 normal range —
# avoids subnormal flush — and the scale is divided back out exactly on the
# host; PSUM accumulation stays fp32).
PACK_DTYPE = "float16"
W_SCALE = 256.0
# RAW: hand-rolled semaphores (no TileContext). Measured SLOWER than Tile
# (52.8 vs 49.3 us): the end-of-program engine butterfly is emitted at the
# bacc level either way, and the raw chunk-granular waits stalled the PE.
RAW = False
# PE_WARM: dummy N=512 matmuls before the real stream to warm the HAM clock
# gate. Measured: no benefit (51.6 vs 49.3 us) — the stream stays DMA-paced.
PE_WARM = 0
# DUAL_RING: alternate input chunks between the SP and ACT HWDGE rings.
# Measured: splits bytes across two queues at the same total rate but delays
# last-chunk completion (48.9 vs 44.5 us) — keep off.
DUAL_RING = False

TEMPERATURE = 0.07
BASE_TEMPERATURE = 1.0
EPS = 1e-12

_BUILT = None          # cached compiled Bass program
LAST_EXEC_NS = None    # set when tracing is enabled (see run_device)


def _build_bass_raw():
    """Raw-bacc build: explicit engine streams + semaphores, no TileContext."""
    import concourse.bacc as bacc
    import concourse.mybir as mybir

    f32 = mybir.dt.float32
    mm_dt = mybir.dt.float16 if PACK_DTYPE == "float16" else f32
    nc = bacc.Bacc(
        "TRN2", target_bir_lowering=False, debug=False, num_devices=N_CORES
    )
    xw = nc.dram_tensor("xw", [128, KTILES, PACK_W], mm_dt, kind="ExternalInput")
    out = nc.dram_tensor("out", [128, B_TOTAL], f32, kind="ExternalOutput")

    nch = len(_CHUNKS)
    nbuf = IO_BUFS
    starts = [0]
    for nk in _CHUNKS:
        starts.append(starts[-1] + nk)
    chunk_bufs = [
        nc.alloc_sbuf_tensor(f"chunk{i}", [128, NK_CHUNK, PACK_W], mm_dt)
        for i in range(nbuf)
    ]
    out_sb = nc.alloc_sbuf_tensor("out_sb", [128, B_TOTAL], f32)
    psum = nc.alloc_psum_tensor("acc", [128, B_TOTAL], f32)

    with (
        nc.semaphore("pe_sem") as pe_sem,
        nc.semaphore("v_sem") as v_sem,
        nc.semaphore("out_sem") as out_sem,
        nc.Block() as block,
    ):
        slot_sems = [nc.alloc_semaphore(f"slot{i}_sem") for i in range(nbuf)]

        @block.sync
        def _(sync):
            for c, nk in enumerate(_CHUNKS):
                if c >= nbuf:
                    # slot reuse: wait until the PE finished chunk c-nbuf
                    sync.wait_ge(pe_sem, c - nbuf + 1)
                buf = chunk_bufs[c % nbuf]
                sync.dma_start(
                    buf[:, :nk, :], xw[:, starts[c] : starts[c] + nk, :]
                ).then_inc(slot_sems[c % nbuf], 16)
            sync.wait_ge(v_sem, 1)
            sync.dma_start(out[:, :], out_sb[:, :]).then_inc(out_sem, 16)
            sync.wait_ge(out_sem, 16)

        @block.tensor
        def _(tensor):
            kt = 0
            for c, nk in enumerate(_CHUNKS):
                tensor.wait_ge(slot_sems[c % nbuf], 16 * (c // nbuf + 1))
                buf = chunk_bufs[c % nbuf]
                for j in range(nk):
                    mm = tensor.matmul(
                        psum[:, :],
                        buf[:, j, B_TOTAL:PACK_W],
                        buf[:, j, 0:B_TOTAL],
                        start=(kt == 0),
                        stop=(kt == KTILES - 1),
                    )
                    kt += 1
                    if j == nk - 1:
                        mm.then_inc(pe_sem, 1)

        @block.vector
        def _(vector):
            vector.wait_ge(pe_sem, nch)
            vector.tensor_copy(out_sb[:, :], psum[:, :]).then_inc(v_sem, 1)

    nc.compile()
    return nc


def _build_bass():
    """Build + compile the per-core Bass program (same program on all cores)."""
    global _BUILT
    if _BUILT is not None:
        return _BUILT
    if RAW:
        _BUILT = _build_bass_raw()
        return _BUILT
    import concourse.bacc as bacc
    import concourse.bass as bass
    import concourse.mybir as mybir
    import concourse.tile as tile

    f32 = mybir.dt.float32
    if PACK_DTYPE == "float16":
        mm_dt = mybir.dt.float16
    else:
        mm_dt = mybir.dt.float32r if MM_F32R else f32
    nc = bacc.Bacc(
        "TRN2", target_bir_lowering=False, debug=False, num_devices=N_CORES
    )
    xw = nc.dram_tensor("xw", [128, KTILES, PACK_W], mm_dt, kind="ExternalInput")
    out = nc.dram_tensor("out", [128, B_TOTAL], f32, kind="ExternalOutput")

    with tile.TileContext(nc) as tc:
        with (
            tc.tile_pool(name="io", bufs=IO_BUFS) as io_pool,
            tc.tile_pool(name="res", bufs=1) as res_pool,
            tc.tile_pool(name="acc", bufs=1, space=bass.MemorySpace.PSUM) as pp,
        ):
            if PE_WARM:
                # dummy matmuls fill the PE's idle window while chunk 0 loads:
                # ~4 us of busy PE flips the HAM clock gate 1.2 -> 2.4 GHz so
                # the real matmul stream runs warm from the start.
                wsrc = io_pool.tile([128, 512], mm_dt, tag="warm")
                nc.gpsimd.memset(wsrc[:, :], 0.0)
                wps = pp.tile([128, 512], f32, tag="warmps")
                for _ in range(PE_WARM):
                    nc.tensor.matmul(
                        wps[:, :], wsrc[:, :128], wsrc[:, :], start=True, stop=True
                    )
            psum = pp.tile([128, B_TOTAL], f32)
            t = 0
            for ci, nk in enumerate(_CHUNKS):
                chunk = io_pool.tile([128, NK_CHUNK, PACK_W], mm_dt, tag="chunk")
                # alternate the two HWDGE rings (SP / ACT) across chunks
                dma_eng = nc.sync if (not DUAL_RING or ci % 2 == 0) else nc.scalar
                dma_eng.dma_start(chunk[:, :nk, :], xw[:, t : t + nk, :])
                for j in range(nk):
                    nc.tensor.matmul(
                        psum[:, :],
                        chunk[:, j, B_TOTAL:PACK_W],   # lhsT: W1 k-tile [128, 128]
                        chunk[:, j, 0:B_TOTAL],        # rhs: X^T k-tile [128, 320]
                        start=(t + j == 0),
                        stop=(t + j == KTILES - 1),
                    )
                t += nk
            out_sb = res_pool.tile([128, B_TOTAL], f32)
            nc.vector.tensor_copy(out_sb[:, :], psum[:, :])
            nc.sync.dma_start(out[:, :], out_sb[:, :])

    nc.compile()
    _BUILT = nc
    return nc


def _pack_inputs(X, W1):
    """X: [320, 120000] f32, W1: [120000, 128] f32 -> 8 per-core packed maps."""
    np_dt = np.float16 if PACK_DTYPE == "float16" else np.float32
    XT = np.ascontiguousarray(X.T).astype(np_dt)  # [120000, 320]
    W1p = (W1 * np.float32(W_SCALE)).astype(np_dt) if np_dt is np.float16 else W1
    in_maps = []
    for c in range(N_CORES):
        ks = c * K_PER_CORE
        ke = ks + K_PER_CORE
        buf = np.zeros((K_PAD, PACK_W), np_dt)
        buf[:K_PER_CORE, :B_TOTAL] = XT[ks:ke]
        buf[:K_PER_CORE, B_TOTAL:] = W1p[ks:ke]
        packed = np.ascontiguousarray(
            buf.reshape(KTILES, 128, PACK_W).transpose(1, 0, 2)
        )
        in_maps.append({"xw": packed})
    return in_maps


def run_device(X, W1, trace=False):
    """Run the sharded partial-GEMM on the 8 NeuronCores; return h_pre [320,128] f64."""
    global LAST_EXEC_NS
    from concourse.bass_utils import run_bass_kernel_spmd

    nc = _build_bass()
    in_maps = _pack_inputs(X, W1)
    # The device occasionally reports NRT_EXEC_UNIT_UNRECOVERABLE on the first
    # execute of a fresh process and recovers on a retry — don't die on it.
    last_exc = None
    for attempt in range(3):
        try:
            res = run_bass_kernel_spmd(
                nc, in_maps, list(range(N_CORES)), trace=trace
            )
            break
        except Exception as e:  # noqa: BLE001
            last_exc = e
            import time

            time.sleep(2.0)
    else:
        raise last_exc
    if res.exec_time_ns is not None:
        LAST_EXEC_NS = res.exec_time_ns
    acc = np.zeros((128, B_TOTAL), np.float64)
    for c in range(N_CORES):
        acc += res.results[c]["out"].astype(np.float64)
    if PACK_DTYPE == "float16":
        acc /= W_SCALE
    return acc.T  # [320, 128] pre-activation (no bias yet)


def _anchor_loss(anchor_e, pos_e, neg_e):
    # mirrors the reference exactly (computed in float64 on host)
    T = TEMPERATURE
    posn = pos_e / np.maximum(
        np.sqrt(np.sum(pos_e * pos_e, axis=-2, keepdims=True)), EPS
    )
    negn = neg_e / np.maximum(
        np.sqrt(np.sum(neg_e * neg_e, axis=-2, keepdims=True)), EPS
    )
    an = anchor_e / np.maximum(np.sqrt(np.sum(anchor_e * anchor_e)), EPS)
    A = (negn @ an) / T
    m = np.max(A)
    log_sum = np.log(np.sum(np.exp(A - m)))
    num = (posn @ an) / T
    return -(T / BASE_TEMPERATURE) * np.mean(num - log_sum)


def _host_loss(E, lab, cf, iff, cc, ic):
    Ef, Ec = E[:NF], E[NF:]
    lc = lab[ic]
    lf = lab[iff]
    wrong_idx = np.nonzero((cc[:, 0] != lc) & (cc[:, 1] == lc))[0]
    corr_idx = np.nonzero(cc[:, 0] == lc)[0]
    corrf_idx = np.nonzero(cf[:, 0] == lf)[0]
    uniq = np.unique(np.concatenate([cc[wrong_idx].ravel(), cc[corr_idx].ravel()]))
    pos_of = {int(c): corrf_idx[cf[corrf_idx, 0] == c] for c in uniq}
    losses = []
    for i in wrong_idx:
        top1, top2 = int(cc[i, 0]), int(cc[i, 1])
        neg_extra = wrong_idx[cc[wrong_idx, 0] == top2]
        neg_e = np.concatenate([Ef[pos_of[top1]], Ec[neg_extra]], axis=0)
        pos_e = Ef[pos_of[top2]]
        if pos_e.shape[0] == 0 or neg_e.shape[0] == 0:
            continue
        losses.append(_anchor_loss(Ec[i], pos_e, neg_e))
    for i in corr_idx:
        pos_e = Ef[pos_of[int(cc[i, 0])]]
        neg_e = Ef[pos_of[int(cc[i, 1])]]
        if pos_e.shape[0] == 0 or neg_e.shape[0] == 0:
            continue
        losses.append(_anchor_loss(Ec[i], pos_e, neg_e))
    if losses:
        return np.mean(np.stack(losses))
    return np.float32(0.0)


def kernel(
    label,
    samples_of_further_pairs,
    class_of_further_pair,
    idx_further_pair,
    samples_of_closest_pairs,
    class_of_closest_pair,
    idx_closest_pair,
    W1,
    b1,
    W2,
    b2,
):
    import os

    X = np.concatenate(
        [
            np.asarray(samples_of_further_pairs, np.float32).reshape(NF, -1),
            np.asarray(samples_of_closest_pairs, np.float32).reshape(NC_SAMPLES, -1),
        ],
        axis=0,
    )  # [320, 120000]
    W1 = np.ascontiguousarray(np.asarray(W1, np.float32))

    h_pre = run_device(X, W1, trace=bool(os.environ.get("KERNEL_TRACE")))
    h = np.maximum(h_pre + np.asarray(b1, np.float64), 0.0)
    E = h @ np.asarray(W2, np.float64) + np.asarray(b2, np.float64)  # [320, 128]

    loss = _host_loss(
        E,
        np.asarray(label).astype(np.int64),
        np.asarray(class_of_further_pair).astype(np.int64),
        np.asarray(idx_further_pair).astype(np.int64),
        np.asarray(class_of_closest_pair).astype(np.int64),
        np.asarray(idx_closest_pair).astype(np.int64),
    )
    return np.asarray(loss, dtype=np.float32)



# revision 2
# speedup vs baseline: 1.9238x; 1.9238x over previous
# Trainium2 Bass kernel for nn_Democracy_loss (supervised-contrastive loss).
#
# Strategy (v2):
#   1. The loss only touches a small subset of the 320 samples: the anchors
#      (wrong/corr closest pairs), neg_extra (subset of wrong anchors), and
#      the pos_of groups (correctly-classified further pairs). The grouping
#      is pure integer metadata available on host BEFORE the GEMM, so we
#      compute the needed row set S (47 rows for the seed-0 inputs) and run
#      the big embed GEMM only for those rows: N shrinks 320 -> |S| (pad 16).
#   2. The dominant cost is streaming X[S] (|S| x 120000) and W1
#      (120000 x 128) from HBM once. We shard the contraction dim K=120000
#      across the 8 cores (15000 rows each) so W1 is NOT replicated; each
#      core computes a partial h_pre^T = W1_c^T @ X_c^T into PSUM and
#      returns it; the host sums the 8 partials.
#   3. Both operands are quantized to fp8 E3M4 (4 mantissa bits, max 15.5):
#      X*2 (|X|<11), W1*512 (sigma 0.003 -> 1.54, |W1|<8.4). The PE
#      accumulates in fp32; host divides by 1024 exactly. Measured final
#      loss rel err ~1.7e-3 (gate 2e-2). Halves DMA bytes vs fp16.
#
# Device layout: per core one packed DRAM input [128, 118, PACK_W] fp8 where
#   packed[p, t, 0:N]        = X[S]^T[k0 + t*128 + p, :]  (moving operand)
#   packed[p, t, N:N+128]    = W1[k0 + t*128 + p, :]      (stationary operand)
# so each chunk of k-tiles is ONE contiguous-per-partition dma_start and each
# k-tile is one matmul: psum[m, n] += W1tile[128k,128m].T @ XTtile[128k,Nn].

import sys

import numpy as np

for _p in ("/opt/trn_rl_repo",):
    if _p not in sys.path:
        sys.path.append(_p)

NF, NC_SAMPLES, B_TOTAL = 256, 64, 320
IN_DIM = 120000
HID = 128
N_CORES = 8
K_PER_CORE = IN_DIM // N_CORES          # 15000
KTILES = (K_PER_CORE + 127) // 128      # 118 (padded to 15104)
K_PAD = KTILES * 128
NK_CHUNK = 10
# Small first chunk -> PE starts earlier; small last chunk -> the final
# matmuls decouple from a full-size DMA.
_CHUNKS = [4] + [NK_CHUNK] * 11 + [4]   # 118 k-tiles
assert sum(_CHUNKS) == KTILES
IO_BUFS = 6

# fp8 E3M4: 1 byte/elem, 4 mantissa bits, max normal 15.5. Scales are exact
# powers of two, divided back out on host. PSUM accumulation is fp32.
PACK_DTYPE = "float8e3"
X_SCALE = 2.0     # X ~ N(0,1): max|X*2| ~ 11 < 15.5
W_SCALE = 512.0   # W1 ~ N(0,0.003): max|W1*512| ~ 8.4 < 15.5
FP8_MAX = 15.5

TEMPERATURE = 0.07
BASE_TEMPERATURE = 1.0
EPS = 1e-12

_BUILT = {}            # n_pad -> compiled Bass program
LAST_EXEC_NS = None    # set when tracing is enabled (see run_device)


def _np_pack_dtype():
    if PACK_DTYPE == "float8e3":
        import ml_dtypes

        return np.dtype(ml_dtypes.float8_e3m4)
    if PACK_DTYPE == "float8e4":
        import ml_dtypes

        return np.dtype(ml_dtypes.float8_e4m3)
    return np.dtype(np.float16)


def _build_bass(n_pad):
    """Build + compile the per-core Bass program (same program on all cores)."""
    if n_pad in _BUILT:
        return _BUILT[n_pad]
    import concourse.bacc as bacc
    import concourse.bass as bass
    import concourse.mybir as mybir
    import concourse.tile as tile

    f32 = mybir.dt.float32
    mm_dt = {
        "float8e3": mybir.dt.float8e3,
        "float8e4": mybir.dt.float8e4,
        "float16": mybir.dt.float16,
    }[PACK_DTYPE]
    pack_w = n_pad + HID
    nc = bacc.Bacc(
        "TRN2", target_bir_lowering=False, debug=False, num_devices=N_CORES
    )
    xw = nc.dram_tensor("xw", [128, KTILES, pack_w], mm_dt, kind="ExternalInput")
    out = nc.dram_tensor("out", [128, n_pad], f32, kind="ExternalOutput")

    with tile.TileContext(nc) as tc:
        with (
            tc.tile_pool(name="io", bufs=IO_BUFS) as io_pool,
            tc.tile_pool(name="res", bufs=1) as res_pool,
            tc.tile_pool(name="acc", bufs=1, space=bass.MemorySpace.PSUM) as pp,
        ):
            psum = pp.tile([128, n_pad], f32)
            t = 0
            for nk in _CHUNKS:
                chunk = io_pool.tile([128, NK_CHUNK, pack_w], mm_dt, tag="chunk")
                nc.sync.dma_start(chunk[:, :nk, :], xw[:, t : t + nk, :])
                for j in range(nk):
                    nc.tensor.matmul(
                        psum[:, :],
                        chunk[:, j, n_pad:pack_w],   # lhsT: W1 k-tile [128, 128]
                        chunk[:, j, 0:n_pad],        # rhs: X^T k-tile [128, n_pad]
                        start=(t + j == 0),
                        stop=(t + j == KTILES - 1),
                    )
                t += nk
            out_sb = res_pool.tile([128, n_pad], f32)
            nc.vector.tensor_copy(out_sb[:, :], psum[:, :])
            nc.sync.dma_start(out[:, :], out_sb[:, :])

    nc.compile()
    _BUILT[n_pad] = nc
    return nc


def _pack_inputs(Xs, W1, n_pad):
    """Xs: [N, 120000] f32 subset, W1: [120000, 128] f32 -> 8 per-core maps."""
    np_dt = _np_pack_dtype()
    pack_w = n_pad + HID
    n = Xs.shape[0]
    if PACK_DTYPE == "float16":
        XTq = Xs.T.astype(np_dt)
        W1q = W1.astype(np_dt)
    else:
        XTq = np.clip(Xs.T * np.float32(X_SCALE), -FP8_MAX, FP8_MAX).astype(np_dt)
        W1q = np.clip(W1 * np.float32(W_SCALE), -FP8_MAX, FP8_MAX).astype(np_dt)
    in_maps = []
    for c in range(N_CORES):
        ks = c * K_PER_CORE
        ke = ks + K_PER_CORE
        buf = np.zeros((K_PAD, pack_w), np_dt)
        buf[:K_PER_CORE, :n] = XTq[ks:ke]
        buf[:K_PER_CORE, n_pad : n_pad + HID] = W1q[ks:ke]
        packed = np.ascontiguousarray(
            buf.reshape(KTILES, 128, pack_w).transpose(1, 0, 2)
        )
        in_maps.append({"xw": packed})
    return in_maps


def run_device(Xs, W1, n_pad, trace=False):
    """Sharded partial-GEMM on the 8 NeuronCores; return h_pre [N,128] f64."""
    global LAST_EXEC_NS
    from concourse.bass_utils import run_bass_kernel_spmd

    nc = _build_bass(n_pad)
    in_maps = _pack_inputs(Xs, W1, n_pad)
    # The device occasionally reports NRT_EXEC_UNIT_UNRECOVERABLE on the first
    # execute of a fresh process and recovers on a retry — don't die on it.
    last_exc = None
    for attempt in range(3):
        try:
            res = run_bass_kernel_spmd(
                nc, in_maps, list(range(N_CORES)), trace=trace
            )
            break
        except Exception as e:  # noqa: BLE001
            last_exc = e
            import time

            time.sleep(2.0)
    else:
        raise last_exc
    if res.exec_time_ns is not None:
        LAST_EXEC_NS = res.exec_time_ns
    acc = np.zeros((128, n_pad), np.float64)
    for c in range(N_CORES):
        acc += res.results[c]["out"].astype(np.float64)
    if PACK_DTYPE != "float16":
        acc /= np.float64(X_SCALE) * np.float64(W_SCALE)
    return acc.T[: Xs.shape[0]]  # [N, 128] pre-activation (no bias yet)


def _anchor_loss(anchor_e, pos_e, neg_e):
    # mirrors the reference exactly (computed in float64 on host)
    T = TEMPERATURE
    posn = pos_e / np.maximum(
        np.sqrt(np.sum(pos_e * pos_e, axis=-2, keepdims=True)), EPS
    )
    negn = neg_e / np.maximum(
        np.sqrt(np.sum(neg_e * neg_e, axis=-2, keepdims=True)), EPS
    )
    an = anchor_e / np.maximum(np.sqrt(np.sum(anchor_e * anchor_e)), EPS)
    A = (negn @ an) / T
    m = np.max(A)
    log_sum = np.log(np.sum(np.exp(A - m)))
    num = (posn @ an) / T
    return -(T / BASE_TEMPERATURE) * np.mean(num - log_sum)


def _grouping(lab, cf, iff, cc, ic):
    """Resolve the ragged grouping; return index arrays (full index space)."""
    lc = lab[ic]
    lf = lab[iff]
    wrong_idx = np.nonzero((cc[:, 0] != lc) & (cc[:, 1] == lc))[0]
    corr_idx = np.nonzero(cc[:, 0] == lc)[0]
    corrf_idx = np.nonzero(cf[:, 0] == lf)[0]
    uniq = np.unique(np.concatenate([cc[wrong_idx].ravel(), cc[corr_idx].ravel()]))
    pos_of = {int(c): corrf_idx[cf[corrf_idx, 0] == c] for c in uniq}
    return wrong_idx, corr_idx, pos_of


def _needed_rows(wrong_idx, corr_idx, pos_of):
    """Rows of the full [320] sample space that the loss actually reads."""
    ef = sorted({int(r) for rows in pos_of.values() for r in rows})
    ec = sorted({int(i) for i in wrong_idx} | {int(i) for i in corr_idx})
    return np.array(ef, np.int64), np.array(ec, np.int64)


def _host_loss(E, lab, cf, iff, cc, ic):
    Ef, Ec = E[:NF], E[NF:]
    wrong_idx, corr_idx, pos_of = _grouping(lab, cf, iff, cc, ic)
    losses = []
    for i in wrong_idx:
        top1, top2 = int(cc[i, 0]), int(cc[i, 1])
        neg_extra = wrong_idx[cc[wrong_idx, 0] == top2]
        neg_e = np.concatenate([Ef[pos_of[top1]], Ec[neg_extra]], axis=0)
        pos_e = Ef[pos_of[top2]]
        if pos_e.shape[0] == 0 or neg_e.shape[0] == 0:
            continue
        losses.append(_anchor_loss(Ec[i], pos_e, neg_e))
    for i in corr_idx:
        pos_e = Ef[pos_of[int(cc[i, 0])]]
        neg_e = Ef[pos_of[int(cc[i, 1])]]
        if pos_e.shape[0] == 0 or neg_e.shape[0] == 0:
            continue
        losses.append(_anchor_loss(Ec[i], pos_e, neg_e))
    if losses:
        return np.mean(np.stack(losses))
    return np.float32(0.0)


def kernel(
    label,
    samples_of_further_pairs,
    class_of_further_pair,
    idx_further_pair,
    samples_of_closest_pairs,
    class_of_closest_pair,
    idx_closest_pair,
    W1,
    b1,
    W2,
    b2,
):
    import os

    lab = np.asarray(label).astype(np.int64)
    cf = np.asarray(class_of_further_pair).astype(np.int64)
    iff = np.asarray(idx_further_pair).astype(np.int64)
    cc = np.asarray(class_of_closest_pair).astype(np.int64)
    ic = np.asarray(idx_closest_pair).astype(np.int64)

    wrong_idx, corr_idx, pos_of = _grouping(lab, cf, iff, cc, ic)
    ef_rows, ec_rows = _needed_rows(wrong_idx, corr_idx, pos_of)
    n_needed = len(ef_rows) + len(ec_rows)
    if n_needed == 0:
        return np.asarray(np.float32(0.0))

    Xf = np.asarray(samples_of_further_pairs, np.float32).reshape(NF, -1)
    Xc = np.asarray(samples_of_closest_pairs, np.float32).reshape(NC_SAMPLES, -1)
    Xs = np.concatenate([Xf[ef_rows], Xc[ec_rows]], axis=0)  # [N, 120000]
    W1 = np.ascontiguousarray(np.asarray(W1, np.float32))
    n_pad = max(16, (n_needed + 15) // 16 * 16)

    h_pre = run_device(Xs, W1, n_pad, trace=bool(os.environ.get("KERNEL_TRACE")))
    h = np.maximum(h_pre + np.asarray(b1, np.float64), 0.0)
    E_sub = h @ np.asarray(W2, np.float64) + np.asarray(b2, np.float64)  # [N, 128]

    # scatter back into the full [320, 128] index space (untouched rows are
    # never read by the loss, by construction of the needed set)
    E = np.zeros((B_TOTAL, HID), np.float64)
    E[ef_rows] = E_sub[: len(ef_rows)]
    E[NF + ec_rows] = E_sub[len(ef_rows) :]

    loss = _host_loss(E, lab, cf, iff, cc, ic)
    return np.asarray(loss, dtype=np.float32)


# revision 7
# speedup vs baseline: 2.2208x; 1.1544x over previous
# Trainium2 Bass kernel for nn_Democracy_loss (supervised-contrastive loss).
#
# Strategy (v2):
#   1. The loss only touches a small subset of the 320 samples: the anchors
#      (wrong/corr closest pairs), neg_extra (subset of wrong anchors), and
#      the pos_of groups (correctly-classified further pairs). The grouping
#      is pure integer metadata available on host BEFORE the GEMM, so we
#      compute the needed row set S (47 rows for the seed-0 inputs) and run
#      the big embed GEMM only for those rows: N shrinks 320 -> |S| (pad 16).
#   2. The dominant cost is streaming X[S] (|S| x 120000) and W1
#      (120000 x 128) from HBM once. We shard the contraction dim K=120000
#      across the 8 cores (15000 rows each) so W1 is NOT replicated; each
#      core computes a partial h_pre^T = W1_c^T @ X_c^T into PSUM and
#      returns it; the host sums the 8 partials.
#   3. Both operands are quantized to fp8 E3M4 (4 mantissa bits, max 15.5):
#      X*2 (|X|<11), W1*512 (sigma 0.003 -> 1.54, |W1|<8.4). The PE
#      accumulates in fp32; host divides by 1024 exactly. Measured final
#      loss rel err ~1.7e-3 (gate 2e-2). Halves DMA bytes vs fp16.
#
# Device layout: per core one packed DRAM input [128, 118, PACK_W] fp8 where
#   packed[p, t, 0:N]        = X[S]^T[k0 + t*128 + p, :]  (moving operand)
#   packed[p, t, N:N+128]    = W1[k0 + t*128 + p, :]      (stationary operand)
# so each chunk of k-tiles is ONE contiguous-per-partition dma_start and each
# k-tile is one matmul: psum[m, n] += W1tile[128k,128m].T @ XTtile[128k,Nn].

import sys

import numpy as np

for _p in ("/opt/trn_rl_repo",):
    if _p not in sys.path:
        sys.path.append(_p)

NF, NC_SAMPLES, B_TOTAL = 256, 64, 320
IN_DIM = 120000
HID = 128
N_CORES = 8
K_PER_CORE = IN_DIM // N_CORES          # 15000
KTILES = (K_PER_CORE + 127) // 128      # 118 (padded to 15104)
K_PAD = KTILES * 128
NK_CHUNK = 10
# Chunk schedule: small first chunk -> PE starts earlier; descending tail ->
# the final matmuls trail the last byte by <0.5us. Few chunks -> few ~650ns
# DMA-issue slices on the Sync queue (the issue rate was the v1 bottleneck).
_CHUNKS = [12, 40, 40, 18, 8]           # 118 k-tiles
assert sum(_CHUNKS) == KTILES
IO_BUFS = 6
# RAW: hand-rolled engine streams + semaphores (no TileContext): saves the
# Tile prologue branch + scheduler barriers and the pool-exit barrier rounds
# in the epilogue, which are inside the graded window.
RAW = True

# fp8 E3M4: 1 byte/elem, 4 mantissa bits, max normal 15.5. Scales are exact
# powers of two, divided back out on host. PSUM accumulation is fp32.
PACK_DTYPE = "float8e3"
X_SCALE = 2.0     # X ~ N(0,1): max|X*2| ~ 11 < 15.5
W_SCALE = 512.0   # W1 ~ N(0,0.003): max|W1*512| ~ 8.4 < 15.5
FP8_MAX = 15.5

TEMPERATURE = 0.07
BASE_TEMPERATURE = 1.0
EPS = 1e-12

_BUILT = {}            # n_pad -> compiled Bass program
LAST_EXEC_NS = None    # set when tracing is enabled (see run_device)
LAST_RESULTS = None    # full BassKernelResults of the last traced run (debug)


def _np_pack_dtype():
    if PACK_DTYPE == "float8e3":
        import ml_dtypes

        return np.dtype(ml_dtypes.float8_e3m4)
    if PACK_DTYPE == "float8e4":
        import ml_dtypes

        return np.dtype(ml_dtypes.float8_e4m3)
    return np.dtype(np.float16)


def _build_bass_raw(n_pad):
    """Raw-bacc build: explicit engine streams + semaphores, no TileContext."""
    import concourse.bacc as bacc
    import concourse.mybir as mybir

    f32 = mybir.dt.float32
    mm_dt = {
        "float8e3": mybir.dt.float8e3,
        "float8e4": mybir.dt.float8e4,
        "float16": mybir.dt.float16,
    }[PACK_DTYPE]
    pack_w = n_pad + HID
    nc = bacc.Bacc(
        "TRN2", target_bir_lowering=False, debug=False, num_devices=N_CORES
    )
    xw = nc.dram_tensor("xw", [128, KTILES, pack_w], mm_dt, kind="ExternalInput")
    out = nc.dram_tensor("out", [128, n_pad], f32, kind="ExternalOutput")

    nch = len(_CHUNKS)
    starts = [0]
    for nk in _CHUNKS:
        starts.append(starts[-1] + nk)
    chunk_bufs = [
        nc.alloc_sbuf_tensor(f"chunk{i}", [128, _CHUNKS[i], pack_w], mm_dt)
        for i in range(nch)
    ]
    out_sb = nc.alloc_sbuf_tensor("out_sb", [128, n_pad], f32)
    psum = nc.alloc_psum_tensor("acc", [128, n_pad], f32)

    with (
        nc.semaphore("pe_sem") as pe_sem,
        nc.semaphore("v_sem") as v_sem,
        nc.semaphore("out_sem") as out_sem,
        nc.Block() as block,
    ):
        slot_sems = [nc.alloc_semaphore(f"slot{i}_sem") for i in range(nch)]

        @block.sync
        def _(sync):
            for c in range(nch):
                sync.dma_start(
                    chunk_bufs[c][:, :, :],
                    xw[:, starts[c] : starts[c + 1], :],
                ).then_inc(slot_sems[c], 16)
            sync.wait_ge(v_sem, 1)
            sync.dma_start(out[:, :], out_sb[:, :]).then_inc(out_sem, 16)
            sync.wait_ge(out_sem, 16)

        @block.tensor
        def _(tensor):
            kt = 0
            for c, nk in enumerate(_CHUNKS):
                tensor.wait_ge(slot_sems[c], 16)
                for j in range(nk):
                    mm = tensor.matmul(
                        psum[:, :],
                        chunk_bufs[c][:, j, n_pad:pack_w],
                        chunk_bufs[c][:, j, 0:n_pad],
                        start=(kt == 0),
                        stop=(kt == KTILES - 1),
                    )
                    kt += 1
            mm.then_inc(pe_sem, 1)

        @block.vector
        def _(vector):
            vector.wait_ge(pe_sem, 1)
            vector.tensor_copy(out_sb[:, :], psum[:, :]).then_inc(v_sem, 1)

    nc.compile()
    return nc


def _build_bass(n_pad):
    """Build + compile the per-core Bass program (same program on all cores)."""
    if n_pad in _BUILT:
        return _BUILT[n_pad]
    if RAW:
        _BUILT[n_pad] = _build_bass_raw(n_pad)
        return _BUILT[n_pad]
    import concourse.bacc as bacc
    import concourse.bass as bass
    import concourse.mybir as mybir
    import concourse.tile as tile

    f32 = mybir.dt.float32
    mm_dt = {
        "float8e3": mybir.dt.float8e3,
        "float8e4": mybir.dt.float8e4,
        "float16": mybir.dt.float16,
    }[PACK_DTYPE]
    pack_w = n_pad + HID
    nc = bacc.Bacc(
        "TRN2", target_bir_lowering=False, debug=False, num_devices=N_CORES
    )
    xw = nc.dram_tensor("xw", [128, KTILES, pack_w], mm_dt, kind="ExternalInput")
    out = nc.dram_tensor("out", [128, n_pad], f32, kind="ExternalOutput")

    with tile.TileContext(nc) as tc:
        with (
            tc.tile_pool(name="io", bufs=IO_BUFS) as io_pool,
            tc.tile_pool(name="res", bufs=1) as res_pool,
            tc.tile_pool(name="acc", bufs=1, space=bass.MemorySpace.PSUM) as pp,
        ):
            psum = pp.tile([128, n_pad], f32)
            t = 0
            nk_max = max(_CHUNKS)
            for nk in _CHUNKS:
                chunk = io_pool.tile([128, nk_max, pack_w], mm_dt, tag="chunk")
                nc.sync.dma_start(chunk[:, :nk, :], xw[:, t : t + nk, :])
                for j in range(nk):
                    nc.tensor.matmul(
                        psum[:, :],
                        chunk[:, j, n_pad:pack_w],   # lhsT: W1 k-tile [128, 128]
                        chunk[:, j, 0:n_pad],        # rhs: X^T k-tile [128, n_pad]
                        start=(t + j == 0),
                        stop=(t + j == KTILES - 1),
                    )
                t += nk
            out_sb = res_pool.tile([128, n_pad], f32)
            nc.vector.tensor_copy(out_sb[:, :], psum[:, :])
            nc.sync.dma_start(out[:, :], out_sb[:, :])

    nc.compile()
    _BUILT[n_pad] = nc
    return nc


def _pack_inputs(Xs, W1, n_pad):
    """Xs: [N, 120000] f32 subset, W1: [120000, 128] f32 -> 8 per-core maps."""
    np_dt = _np_pack_dtype()
    pack_w = n_pad + HID
    n = Xs.shape[0]
    if PACK_DTYPE == "float16":
        XTq = Xs.T.astype(np_dt)
        W1q = W1.astype(np_dt)
    else:
        XTq = np.clip(Xs.T * np.float32(X_SCALE), -FP8_MAX, FP8_MAX).astype(np_dt)
        W1q = np.clip(W1 * np.float32(W_SCALE), -FP8_MAX, FP8_MAX).astype(np_dt)
    in_maps = []
    for c in range(N_CORES):
        ks = c * K_PER_CORE
        ke = ks + K_PER_CORE
        buf = np.zeros((K_PAD, pack_w), np_dt)
        buf[:K_PER_CORE, :n] = XTq[ks:ke]
        buf[:K_PER_CORE, n_pad : n_pad + HID] = W1q[ks:ke]
        packed = np.ascontiguousarray(
            buf.reshape(KTILES, 128, pack_w).transpose(1, 0, 2)
        )
        in_maps.append({"xw": packed})
    return in_maps


def run_device(Xs, W1, n_pad, trace=False):
    """Sharded partial-GEMM on the 8 NeuronCores; return h_pre [N,128] f64."""
    global LAST_EXEC_NS
    from concourse.bass_utils import run_bass_kernel_spmd

    nc = _build_bass(n_pad)
    in_maps = _pack_inputs(Xs, W1, n_pad)
    # The device occasionally reports NRT_EXEC_UNIT_UNRECOVERABLE on the first
    # execute of a fresh process and recovers on a retry — don't die on it.
    last_exc = None
    for attempt in range(3):
        try:
            res = run_bass_kernel_spmd(
                nc, in_maps, list(range(N_CORES)), trace=trace
            )
            break
        except Exception as e:  # noqa: BLE001
            last_exc = e
            import time

            time.sleep(2.0)
    else:
        raise last_exc
    if res.exec_time_ns is not None:
        LAST_EXEC_NS = res.exec_time_ns
    if trace:
        global LAST_RESULTS
        LAST_RESULTS = res
    acc = np.zeros((128, n_pad), np.float64)
    for c in range(N_CORES):
        acc += res.results[c]["out"].astype(np.float64)
    if PACK_DTYPE != "float16":
        acc /= np.float64(X_SCALE) * np.float64(W_SCALE)
    return acc.T[: Xs.shape[0]]  # [N, 128] pre-activation (no bias yet)


def _anchor_loss(anchor_e, pos_e, neg_e):
    # mirrors the reference exactly (computed in float64 on host)
    T = TEMPERATURE
    posn = pos_e / np.maximum(
        np.sqrt(np.sum(pos_e * pos_e, axis=-2, keepdims=True)), EPS
    )
    negn = neg_e / np.maximum(
        np.sqrt(np.sum(neg_e * neg_e, axis=-2, keepdims=True)), EPS
    )
    an = anchor_e / np.maximum(np.sqrt(np.sum(anchor_e * anchor_e)), EPS)
    A = (negn @ an) / T
    m = np.max(A)
    log_sum = np.log(np.sum(np.exp(A - m)))
    num = (posn @ an) / T
    return -(T / BASE_TEMPERATURE) * np.mean(num - log_sum)


def _grouping(lab, cf, iff, cc, ic):
    """Resolve the ragged grouping; return index arrays (full index space)."""
    lc = lab[ic]
    lf = lab[iff]
    wrong_idx = np.nonzero((cc[:, 0] != lc) & (cc[:, 1] == lc))[0]
    corr_idx = np.nonzero(cc[:, 0] == lc)[0]
    corrf_idx = np.nonzero(cf[:, 0] == lf)[0]
    uniq = np.unique(np.concatenate([cc[wrong_idx].ravel(), cc[corr_idx].ravel()]))
    pos_of = {int(c): corrf_idx[cf[corrf_idx, 0] == c] for c in uniq}
    return wrong_idx, corr_idx, pos_of


def _needed_rows(wrong_idx, corr_idx, pos_of):
    """Rows of the full [320] sample space that the loss actually reads."""
    ef = sorted({int(r) for rows in pos_of.values() for r in rows})
    ec = sorted({int(i) for i in wrong_idx} | {int(i) for i in corr_idx})
    return np.array(ef, np.int64), np.array(ec, np.int64)


def _host_loss(E, lab, cf, iff, cc, ic):
    Ef, Ec = E[:NF], E[NF:]
    wrong_idx, corr_idx, pos_of = _grouping(lab, cf, iff, cc, ic)
    losses = []
    for i in wrong_idx:
        top1, top2 = int(cc[i, 0]), int(cc[i, 1])
        neg_extra = wrong_idx[cc[wrong_idx, 0] == top2]
        neg_e = np.concatenate([Ef[pos_of[top1]], Ec[neg_extra]], axis=0)
        pos_e = Ef[pos_of[top2]]
        if pos_e.shape[0] == 0 or neg_e.shape[0] == 0:
            continue
        losses.append(_anchor_loss(Ec[i], pos_e, neg_e))
    for i in corr_idx:
        pos_e = Ef[pos_of[int(cc[i, 0])]]
        neg_e = Ef[pos_of[int(cc[i, 1])]]
        if pos_e.shape[0] == 0 or neg_e.shape[0] == 0:
            continue
        losses.append(_anchor_loss(Ec[i], pos_e, neg_e))
    if losses:
        return np.mean(np.stack(losses))
    return np.float32(0.0)


def kernel(
    label,
    samples_of_further_pairs,
    class_of_further_pair,
    idx_further_pair,
    samples_of_closest_pairs,
    class_of_closest_pair,
    idx_closest_pair,
    W1,
    b1,
    W2,
    b2,
):
    import os

    lab = np.asarray(label).astype(np.int64)
    cf = np.asarray(class_of_further_pair).astype(np.int64)
    iff = np.asarray(idx_further_pair).astype(np.int64)
    cc = np.asarray(class_of_closest_pair).astype(np.int64)
    ic = np.asarray(idx_closest_pair).astype(np.int64)

    wrong_idx, corr_idx, pos_of = _grouping(lab, cf, iff, cc, ic)
    ef_rows, ec_rows = _needed_rows(wrong_idx, corr_idx, pos_of)
    n_needed = len(ef_rows) + len(ec_rows)
    if n_needed == 0:
        return np.asarray(np.float32(0.0))

    Xf = np.asarray(samples_of_further_pairs, np.float32).reshape(NF, -1)
    Xc = np.asarray(samples_of_closest_pairs, np.float32).reshape(NC_SAMPLES, -1)
    Xs = np.concatenate([Xf[ef_rows], Xc[ec_rows]], axis=0)  # [N, 120000]
    W1 = np.ascontiguousarray(np.asarray(W1, np.float32))
    n_pad = max(16, (n_needed + 15) // 16 * 16)

    h_pre = run_device(Xs, W1, n_pad, trace=bool(os.environ.get("KERNEL_TRACE")))
    h = np.maximum(h_pre + np.asarray(b1, np.float64), 0.0)
    E_sub = h @ np.asarray(W2, np.float64) + np.asarray(b2, np.float64)  # [N, 128]

    # scatter back into the full [320, 128] index space (untouched rows are
    # never read by the loss, by construction of the needed set)
    E = np.zeros((B_TOTAL, HID), np.float64)
    E[ef_rows] = E_sub[: len(ef_rows)]
    E[NF + ec_rows] = E_sub[len(ef_rows) :]

    loss = _host_loss(E, lab, cf, iff, cc, ic)
    return np.asarray(loss, dtype=np.float32)
